# revision 33
# baseline (speedup 1.0000x reference)
"""Multi-head cross-attention Trainium2 kernel (8 NeuronCores, SPMD).

Problem: nn_MultiHeadCrossAttention_31791347925263
  x:[4,2048,768], y:[4,2048,768], 12 heads x 64, fp32.
  out = softmax((x Wq^T)(y Wk^T)^T / 8 + mask) (y Wv^T) Wo^T   (+ zero biases)

Sharding: 8 cores = (batch b in 0..3) x (head-half g in 0..1). Each core
computes 6 heads over ALL 2048 queries x 2048 keys of its batch, plus its
partial output projection (Wo rows for its heads); the host sums the two
partials per batch during unshard (the "all-reduce after the output
projection" of the tensor-parallel head split, done for free off-device).
Vs. a query-split this halves the k/v/q projection work per core.

Design (v7, ACT-limited pipeline, all-bf16):
  Measured HW laws driving this shape:
  - A K=128 matmul streams 512 moving cols at ~full clock; K<=64
    matmuls run exactly 2x slower. So QK (contraction = head_dim = 64)
    uses ZERO-PADDED stationaries: kTz[h] is [128, 2048] with the
    head's k in the same 64 partition rows its q occupies in the packed
    qT block, zeros in the other 64 (killing the sibling head's q).
  - fp8 anywhere in the PV chain costs ~2-3% output error (softmax
    output rel err ~= per-element rel err of P~/v) and fp8 DoubleRow
    is NOT faster per output column on this HW -> everything bf16.
  - The Scalar-engine exp (25.2M scores -> 192 x [128,1024] ACTIVATEs
    at ~1.12us) is a ~214us floor; PE work (541k cols ~= 225us) hides
    under it: the attention stream (QK 2mm + PV 2mm per (head, skb)
    unit) is interleaved with projection chunks (6-mm units) injected
    into the slack and force-drained JIT before their consumers.
  - The 2048 queries are processed as two 1024-query passes (PSUM slot
    size); pass 0's partial output projection injects into pass 1's
    attention, so only pass 1's remains in the tail.
  - PSUM: 2x QK score slots [128,1024] (4 banks) + PV accumulator
    [65,1024] (2) + 2 projection slots [128,512] (2).
  - PV's 65th stationary column (ones) accumulates the softmax
    denominator free; normalize = DVE copies + reciprocal_approx_fast
    (needs a partition-0-aligned source, hence the den bounce) +
    gpsimd partition-broadcast + DVE mul into bf16 vnorm tiles.
"""

import numpy as np

B, S, D = 4, 2048, 768
H, Dh = 12, 64
HC = H // 2          # 6 heads per core
DC = HC * Dh         # 384 projected dims per core
SQ = 1024            # queries per pipeline pass (2 passes = all 2048)
N_CORES = 8
DB = D // 128        # 6 d_model contraction blocks
OB = DC // 128       # 3 output blocks for q/k per core
SKB = S // 128       # 16 key blocks
VPW = HC * (Dh + 1)  # 390: v' width (64 v cols + 1 ones col per head)

_cache = {}


def _build_nc():
    import concourse.mybir as mybir
    import concourse.tile as tile
    from concourse import bacc

    f32 = mybir.dt.float32
    bf16 = mybir.dt.bfloat16
    EXP = mybir.ActivationFunctionType.Exp

    nc = bacc.Bacc("TRN2", target_bir_lowering=False)
    x16 = nc.dram_tensor("x16", [128, DB, S], bf16, kind="ExternalInput")
    y16 = nc.dram_tensor("y16", [128, DB, S], bf16, kind="ExternalInput")
    wq16 = nc.dram_tensor("wq16", [128, DB, DC], bf16, kind="ExternalInput")
    wk16 = nc.dram_tensor("wk16", [128, DB, DC], bf16, kind="ExternalInput")
    wv16 = nc.dram_tensor("wv16", [128, DB, DC], bf16, kind="ExternalInput")
    wo16 = nc.dram_tensor("wo16", [128, OB, D], bf16, kind="ExternalInput")
    out = nc.dram_tensor("out", [S, D], f32, kind="ExternalOutput")

    with tile.TileContext(nc) as tc:
        with tc.tile_pool(name="persist", bufs=1) as pp, \
             tc.tile_pool(name="mmps", bufs=2, space="PSUM") as mm_ps, \
             tc.tile_pool(name="vtps", bufs=1, space="PSUM") as vt_ps, \
             tc.tile_pool(name="pjps", bufs=2, space="PSUM") as pj_ps, \
             tc.tile_pool(name="pt16p", bufs=6) as pt_pool, \
             tc.tile_pool(name="nrm", bufs=1) as nrm_pool, \
             tc.tile_pool(name="osb", bufs=3) as o_pool:

            x16t = pp.tile([128, DB, S], bf16, name="x16t")
            wq16t = pp.tile([128, DB, DC], bf16, name="wq16t")
            y16t = pp.tile([128, DB, S], bf16, name="y16t")
            wk16t = pp.tile([128, DB, DC], bf16, name="wk16t")
            wv16t = pp.tile([128, DB, DC], bf16, name="wv16t")
            wo16t = pp.tile([128, OB, D], bf16, name="wo16t")

            # zero-padded per-head k: head h's k in the same 64 partition
            # rows its q occupies in qT (even: 0-63, odd: 64-127), zeros
            # in the other 64
            kTz = [pp.tile([128, S], bf16, name=f"kTz{i}") for i in range(HC)]
            qT = [pp.tile([128, S], bf16, name=f"qT{i}") for i in range(OB)]
            # vnorm[qhalf][ob]: normalized values for that query half
            vnorm = [[pp.tile([128, SQ], bf16, name=f"vn{g}_{i}")
                      for i in range(OB)] for g in range(2)]
            vp16 = [pp.tile([128, VPW], bf16, name=f"vp16_{i}")
                    for i in range(SKB)]
            vp3 = [t.rearrange("p (h c) -> p h c", c=Dh + 1) for t in vp16]

            # ---- input DMA, priority order ----
            nc.sync.dma_start(out=wk16t, in_=wk16[:, :, :])
            for c4 in range(4):
                for kb in range(DB):
                    cs = slice(c4 * 512, (c4 + 1) * 512)
                    nc.sync.dma_start(out=y16t[:, kb, cs],
                                      in_=y16[:, kb, cs])
                if c4 == 0:
                    nc.sync.dma_start(out=wq16t, in_=wq16[:, :, :])
                for kb in range(DB):
                    cs = slice(c4 * 512, (c4 + 1) * 512)
                    nc.sync.dma_start(out=x16t[:, kb, cs],
                                      in_=x16[:, kb, cs])
            nc.sync.dma_start(out=wv16t, in_=wv16[:, :, :])
            nc.sync.dma_start(out=wo16t, in_=wo16[:, :, :])

            for h in range(HC):
                z0 = 64 if h % 2 == 0 else 0
                nc.gpsimd.memset(kTz[h][z0:z0 + 64, :], 0.0)
            for skb in range(SKB):
                nc.vector.memset(vp3[skb][:, :, Dh], 1.0)

            # ---- projection / output chunk emitters ----
            def emit_kt_chunk(ob, c4):
                ps = pj_ps.tile([128, 512], f32, name="pjps", tag="pjps")
                for kb in range(DB):
                    nc.tensor.matmul(
                        ps[:, :],
                        wk16t[:, kb, ob * 128:(ob + 1) * 128],
                        y16t[:, kb, c4 * 512:(c4 + 1) * 512],
                        start=(kb == 0), stop=(kb == DB - 1))
                cols = slice(c4 * 512, (c4 + 1) * 512)
                nc.vector.tensor_copy(kTz[2 * ob][0:64, cols], ps[0:64, :])
                nc.vector.tensor_copy(kTz[2 * ob + 1][64:128, cols],
                                      ps[64:128, :])

            def emit_qt_chunk(ob, c4):
                ps = pj_ps.tile([128, 512], f32, name="pjps", tag="pjps")
                for kb in range(DB):
                    nc.tensor.matmul(
                        ps[:, :],
                        wq16t[:, kb, ob * 128:(ob + 1) * 128],
                        x16t[:, kb, c4 * 512:(c4 + 1) * 512],
                        start=(kb == 0), stop=(kb == DB - 1))
                nc.vector.tensor_copy(
                    qT[ob][:, c4 * 512:(c4 + 1) * 512], ps[:, :])

            def emit_vp_chunk(skb):
                ps = pj_ps.tile([128, 512], f32, name="pjps", tag="pjps")
                for kb in range(DB):
                    nc.tensor.matmul(
                        ps[:, 0:DC],
                        y16t[:, kb, skb * 128:(skb + 1) * 128],
                        wv16t[:, kb, :],
                        start=(kb == 0), stop=(kb == DB - 1))
                src = ps[:, 0:DC].rearrange("p (h c) -> p h c", c=Dh)
                nc.vector.tensor_copy(vp3[skb][:, :, 0:Dh], src)

            o_stage = {}

            def emit_o_half(g, sqb, nc2):
                # half of a partial output-projection query block, using a
                # 1-bank pj slot (safe to interleave with live attention)
                n0, n1 = nc2 * 512, min(D, (nc2 + 1) * 512)
                ps = pj_ps.tile([128, 512], f32, name="pjps", tag="pjps")
                for kb in range(OB):
                    nc.tensor.matmul(
                        ps[:, 0:n1 - n0],
                        vnorm[g][kb][:, sqb * 128:(sqb + 1) * 128],
                        wo16t[:, kb, n0:n1],
                        start=(kb == 0), stop=(kb == OB - 1))
                if nc2 == 0:
                    o_stage[(g, sqb)] = o_pool.tile([128, D], f32,
                                                    name="osb")
                ot = o_stage[(g, sqb)]
                nc.vector.tensor_copy(ot[:, n0:n1], ps[:, 0:n1 - n0])
                if nc2 == 1:
                    o_stage.pop((g, sqb))
                    row0 = g * SQ + sqb * 128
                    nc.sync.dma_start(out=out[row0:row0 + 128, :],
                                      in_=ot[:, :])

            def emit_o_chunk(g, sqb):
                # tail-time version: big rotating slots (mm/valT free then)
                if sqb % 3 < 2:
                    op = mm_ps.tile([128, D], f32, name="mmps", tag="mmps",
                                    padded_shape=[128, SQ])
                else:
                    op = vt_ps.tile([128, D], f32, name="valT", tag="valT",
                                    padded_shape=[128, SQ])
                for kb in range(OB):
                    for nc2 in range(2):
                        n0, n1 = nc2 * 512, min(D, (nc2 + 1) * 512)
                        nc.tensor.matmul(
                            op[:, n0:n1],
                            vnorm[g][kb][:, sqb * 128:(sqb + 1) * 128],
                            wo16t[:, kb, n0:n1],
                            start=(kb == 0), stop=(kb == OB - 1))
                ot = o_pool.tile([128, D], f32, name="osb")
                nc.vector.tensor_copy(ot[:, :], op[:, :])
                row0 = g * SQ + sqb * 128
                nc.sync.dma_start(out=out[row0:row0 + 128, :], in_=ot[:, :])

            # task queue: (tag, mm_count, emit_fn), in need-by order
            tasks = []

            def queue_ob(ob):
                for c4 in range(4):
                    tasks.append((("kt", ob, c4), 6,
                                  lambda ob=ob, c4=c4: emit_kt_chunk(ob, c4)))

            for skb in range(SKB):
                tasks.append((("vp", skb), 6,
                              lambda skb=skb: emit_vp_chunk(skb)))
            queue_ob(1)
            for c4 in range(2, 4):  # qT[0] cols 1024-2047 (pass 1)
                tasks.append((("qt", 0, c4), 6,
                              lambda c4=c4: emit_qt_chunk(0, c4)))
            queue_ob(2)
            for ob in (1, 2):
                for c4 in range(4):
                    tasks.append((("qt", ob, c4), 6,
                                  lambda ob=ob, c4=c4: emit_qt_chunk(ob, c4)))

            state = {"budget": 0.0}

            def force(pred):
                rest = []
                for t in tasks:
                    if pred(t[0]):
                        t[2]()
                        state["budget"] -= t[1]
                    else:
                        rest.append(t)
                tasks[:] = rest

            def inject(budget_add):
                state["budget"] += budget_add
                while tasks and tasks[0][1] <= state["budget"]:
                    tag, mms, fn = tasks.pop(0)
                    fn()
                    state["budget"] -= mms

            # ---- prelude: kTz[0,1], then qT[0] cols 0-1023 ----
            for c4 in range(4):
                emit_kt_chunk(0, c4)
            for c4 in range(2):
                emit_qt_chunk(0, c4)

            # ---- attention pipeline over 12 (qhalf, head) pseudo-units ----
            pt_live = {}

            def emit_qk(g, h, s):
                hb = h // 2
                st = mm_ps.tile([128, SQ], f32, name="mmps", tag="mmps",
                                padded_shape=[128, SQ])
                for j in range(2):
                    q0 = g * SQ + j * 512
                    nc.tensor.matmul(
                        st[:, j * 512:(j + 1) * 512],
                        kTz[h][:, s * 128:(s + 1) * 128],
                        qT[hb][:, q0:q0 + 512],
                        start=True, stop=True)
                pt = pt_pool.tile([128, SQ], bf16, name="pt16")
                nc.scalar.activation(pt[:, :], st[:, :], EXP, scale=0.125)
                pt_live[(g, h, s)] = pt

            vt_live = {}

            def emit_pv(g, h, s):
                if s == 0:
                    vt_live[(g, h)] = vt_ps.tile([65, SQ], f32, name="valT",
                                                 tag="valT",
                                                 padded_shape=[128, SQ])
                vt = vt_live[(g, h)]
                pt = pt_live.pop((g, h, s))
                force(lambda t: t[0] == "vp" and t[1] == s)
                for j in range(2):
                    nc.tensor.matmul(
                        vt[:, j * 512:(j + 1) * 512],
                        vp16[s][:, h * 65:h * 65 + 65],
                        pt[:, j * 512:(j + 1) * 512],
                        start=(s == 0), stop=(s == SKB - 1))

            def emit_vnorm(g, h):
                hb, r0 = h // 2, (h % 2) * 64
                vt = vt_live.pop((g, h))
                vals = nrm_pool.tile([64, SQ], f32, name="vals")
                nc.vector.tensor_copy(vals[:, :], vt[0:64, :])
                den = nrm_pool.tile([1, SQ], f32, name="den")
                nc.vector.tensor_copy(den[:, :], vt[64:65, :])
                rec = nrm_pool.tile([1, SQ], f32, name="rec")
                # denominators are positive, well inside normal fp32 range;
                # approx-fast needs a partition-aligned source (den bounce)
                nc.vector.reciprocal_approx_fast(rec[:, :], den[:, :])
                rbc = nrm_pool.tile([64, SQ], f32, name="rbc")
                nc.gpsimd.partition_broadcast(rbc[:, :], rec[:, :])
                nc.vector.tensor_mul(
                    vnorm[g][hb][r0:r0 + 64, :], vals[:, :], rbc[:, :])

            units = [(g, h) for g in range(2) for h in range(HC)]
            NS = len(units) * SKB  # 192
            LAG = 3
            for u in range(NS + LAG):
                if u < NS:
                    uh, s2 = divmod(u, SKB)
                    g2, h2 = units[uh]
                    if s2 == 0 and h2 % 2 == 0:
                        # qT block hb2's columns for this query half
                        force(lambda t, hb2=h2 // 2, g=g2:
                              t[0] == "qt" and t[1] == hb2
                              and t[2] // 2 == g)
                    if s2 % 4 == 0:
                        force(lambda t, hb2=h2 // 2, c4=s2 // 4:
                              t[0] == "kt" and t[1] == hb2 and t[2] == c4)
                    emit_qk(g2, h2, s2)
                    inject(0.95)
                if u >= LAG:
                    uh, s1 = divmod(u - LAG, SKB)
                    g1, h1 = units[uh]
                    emit_pv(g1, h1, s1)
                    if s1 == SKB - 1:
                        emit_vnorm(g1, h1)
                        if h1 == HC - 1 and g1 == 0:
                            # pass 0 finished: queue its output projection
                            for sqb in range(8):
                                for nc2 in range(2):
                                    tasks.append((("o0", sqb), 3,
                                                  lambda sqb=sqb, nc2=nc2:
                                                  emit_o_half(0, sqb, nc2)))
                    inject(0.95)

            force(lambda t: True)

            # ---- pass 1 output projection ----
            for sqb in range(8):
                emit_o_chunk(1, sqb)

    nc.compile()
    return nc


def _get_nc():
    if "nc" not in _cache:
        _cache["nc"] = _build_nc()
    return _cache["nc"]


def _host_fallback(x, y, mask, Wq, bq, Wkv, bkv, Wo, bo):
    Bb, Ss, _ = x.shape
    q = x @ Wq.T + bq
    kv = y @ Wkv.T + bkv
    q = q.reshape(Bb, Ss, H, Dh).transpose(0, 2, 1, 3)
    kv = kv.reshape(Bb, Ss, H, 2 * Dh).transpose(0, 2, 1, 3)
    k, v = kv[..., :Dh], kv[..., Dh:]
    scaled = np.einsum("bhqd,bhkd->bhqk", q, k) / np.sqrt(np.float32(Dh))
    scaled = scaled + mask
    scaled -= scaled.max(axis=-1, keepdims=True)
    e = np.exp(scaled)
    attn = e / e.sum(axis=-1, keepdims=True)
    values = np.einsum("bhqk,bhkd->bhqd", attn, v)
    values = values.transpose(0, 2, 1, 3).reshape(Bb, Ss, H * Dh)
    return (values @ Wo.T + bo).astype(np.float32)


def _blk(mat_t, dtype):
    """[768, N] row-blocked to [128, 6, N] in the given ml dtype."""
    n = mat_t.shape[1]
    return np.ascontiguousarray(
        mat_t.reshape(-1, 128, n).transpose(1, 0, 2)).astype(dtype)


def _run(inputs, trace=False, trace_cores=None):
    """Returns (full_output, BassKernelResults)."""
    import ml_dtypes
    from concourse.bass_utils import run_bass_kernel_spmd

    bf16 = ml_dtypes.bfloat16

    x = np.ascontiguousarray(np.asarray(inputs["x"], dtype=np.float32))
    y = np.ascontiguousarray(np.asarray(inputs["y"], dtype=np.float32))
    Wq = np.asarray(inputs["Wq"], dtype=np.float32)
    Wkv = np.asarray(inputs["Wkv"], dtype=np.float32)
    Wo = np.asarray(inputs["Wo"], dtype=np.float32)

    # Reference reshapes kv to [B,S,H,2*Dh]: per head, rows h*128..h*128+63 of
    # Wkv are the k-projection, rows h*128+64..h*128+127 the v-projection.
    in_maps = []
    for c in range(N_CORES):
        b, g = c // 2, c % 2
        heads = range(g * HC, (g + 1) * HC)
        k_rows = np.concatenate([np.arange(h * 128, h * 128 + Dh)
                                 for h in heads])
        v_rows = np.concatenate([np.arange(h * 128 + Dh, (h + 1) * 128)
                                 for h in heads])
        q_rows = np.concatenate([np.arange(h * Dh, (h + 1) * Dh)
                                 for h in heads])
        in_maps.append({
            "x16": _blk(x[b].T, bf16),
            "y16": _blk(y[b].T, bf16),
            "wq16": _blk(Wq[q_rows].T, bf16),
            "wk16": _blk(Wkv[k_rows].T, bf16),
            "wv16": _blk(Wkv[v_rows].T, bf16),
            "wo16": _blk(Wo[:, q_rows].T, bf16),
        })

    nc = _get_nc()
    res = run_bass_kernel_spmd(nc, in_maps, core_ids=list(range(N_CORES)),
                               trace=trace, trace_cores=trace_cores)
    out = np.empty((B, S, D), dtype=np.float32)
    for b in range(B):
        # host-side "all-reduce" of the two head-halves' partial projections
        out[b] = res.results[2 * b]["out"]
        out[b] += res.results[2 * b + 1]["out"]
    return out, res


def kernel(**inputs) -> np.ndarray:
    mask = np.asarray(inputs["mask"], dtype=np.float32)
    bq = np.asarray(inputs["bq"], dtype=np.float32)
    bkv = np.asarray(inputs["bkv"], dtype=np.float32)
    bo = np.asarray(inputs["bo"], dtype=np.float32)
    if mask.any() or bq.any() or bkv.any() or bo.any():
        # Device kernel hardcodes zero mask/biases; stay correct regardless.
        return _host_fallback(
            np.asarray(inputs["x"], dtype=np.float32),
            np.asarray(inputs["y"], dtype=np.float32),
            mask, np.asarray(inputs["Wq"], dtype=np.float32), bq,
            np.asarray(inputs["Wkv"], dtype=np.float32), bkv,
            np.asarray(inputs["Wo"], dtype=np.float32), bo)
    out, _ = _run(inputs)
    return out


# revision 34
# speedup vs baseline: 1.0687x; 1.0687x over previous
"""Multi-head cross-attention Trainium2 kernel (8 NeuronCores, SPMD).

Problem: nn_MultiHeadCrossAttention_31791347925263
  x:[4,2048,768], y:[4,2048,768], 12 heads x 64, fp32.
  out = softmax((x Wq^T)(y Wk^T)^T / 8 + mask) (y Wv^T) Wo^T   (+ zero biases)

Sharding: 8 cores = (batch b in 0..3) x (head-half g in 0..1). Each core
computes 6 heads over ALL 2048 queries x 2048 keys of its batch, plus its
partial output projection (Wo rows for its heads); the host sums the two
partials per batch during unshard (the "all-reduce after the output
projection" of the tensor-parallel head split, done for free off-device).
Vs. a query-split this halves the k/v/q projection work per core.

Design (v7, ACT-limited pipeline, all-bf16):
  Measured HW laws driving this shape:
  - A K=128 matmul streams 512 moving cols at ~full clock; K<=64
    matmuls run exactly 2x slower. So QK (contraction = head_dim = 64)
    uses ZERO-PADDED stationaries: kTz[h] is [128, 2048] with the
    head's k in the same 64 partition rows its q occupies in the packed
    qT block, zeros in the other 64 (killing the sibling head's q).
  - fp8 anywhere in the PV chain costs ~2-3% output error (softmax
    output rel err ~= per-element rel err of P~/v) and fp8 DoubleRow
    is NOT faster per output column on this HW -> everything bf16.
  - The Scalar-engine exp (25.2M scores -> 192 x [128,1024] ACTIVATEs
    at ~1.12us) is a ~214us floor; PE work (541k cols ~= 225us) hides
    under it: the attention stream (QK 2mm + PV 2mm per (head, skb)
    unit) is interleaved with projection chunks (6-mm units) injected
    into the slack and force-drained JIT before their consumers.
  - The 2048 queries are processed as two 1024-query passes (PSUM slot
    size); pass 0's partial output projection injects into pass 1's
    attention, so only pass 1's remains in the tail.
  - PSUM: 2x QK score slots [128,1024] (4 banks) + PV accumulator
    [65,1024] (2) + 2 projection slots [128,512] (2).
  - PV's 65th stationary column (ones) accumulates the softmax
    denominator free; normalize = DVE copies + reciprocal_approx_fast
    (needs a partition-0-aligned source, hence the den bounce) +
    gpsimd partition-broadcast + DVE mul into bf16 vnorm tiles.
"""

import numpy as np

B, S, D = 4, 2048, 768
H, Dh = 12, 64
HC = H // 2          # 6 heads per core
DC = HC * Dh         # 384 projected dims per core
SQ = 1024            # queries per pipeline pass (2 passes = all 2048)
N_CORES = 8
DB = D // 128        # 6 d_model contraction blocks
OB = DC // 128       # 3 output blocks for q/k per core
SKB = S // 128       # 16 key blocks
VPW = HC * (Dh + 1)  # 390: v' width (64 v cols + 1 ones col per head)

_cache = {}


def _build_nc():
    import concourse.mybir as mybir
    import concourse.tile as tile
    from concourse import bacc

    f32 = mybir.dt.float32
    bf16 = mybir.dt.bfloat16
    EXP = mybir.ActivationFunctionType.Exp

    nc = bacc.Bacc("TRN2", target_bir_lowering=False)
    x16 = nc.dram_tensor("x16", [128, DB, S], bf16, kind="ExternalInput")
    y16 = nc.dram_tensor("y16", [128, DB, S], bf16, kind="ExternalInput")
    wq16 = nc.dram_tensor("wq16", [128, DB, DC], bf16, kind="ExternalInput")
    wk16 = nc.dram_tensor("wk16", [128, DB, DC], bf16, kind="ExternalInput")
    wv16 = nc.dram_tensor("wv16", [128, DB, DC], bf16, kind="ExternalInput")
    wo16 = nc.dram_tensor("wo16", [128, OB, D], bf16, kind="ExternalInput")
    out = nc.dram_tensor("out", [S, D], f32, kind="ExternalOutput")

    with tile.TileContext(nc) as tc:
        with tc.tile_pool(name="persist", bufs=1) as pp, \
             tc.tile_pool(name="mmps", bufs=2, space="PSUM") as mm_ps, \
             tc.tile_pool(name="vtps", bufs=1, space="PSUM") as vt_ps, \
             tc.tile_pool(name="pjps", bufs=2, space="PSUM") as pj_ps, \
             tc.tile_pool(name="pt16p", bufs=6) as pt_pool, \
             tc.tile_pool(name="nrm", bufs=1) as nrm_pool, \
             tc.tile_pool(name="osb", bufs=3) as o_pool:

            x16t = pp.tile([128, DB, S], bf16, name="x16t")
            wq16t = pp.tile([128, DB, DC], bf16, name="wq16t")
            y16t = pp.tile([128, DB, S], bf16, name="y16t")
            wk16t = pp.tile([128, DB, DC], bf16, name="wk16t")
            wv16t = pp.tile([128, DB, DC], bf16, name="wv16t")
            wo16t = pp.tile([128, OB, D], bf16, name="wo16t")

            # zero-padded per-head k: head h's k in the same 64 partition
            # rows its q occupies in qT (even: 0-63, odd: 64-127), zeros
            # in the other 64
            kTz = [pp.tile([128, S], bf16, name=f"kTz{i}") for i in range(HC)]
            qT = [pp.tile([128, S], bf16, name=f"qT{i}") for i in range(OB)]
            # vnorm[qhalf][ob]: normalized values for that query half
            vnorm = [[pp.tile([128, SQ], bf16, name=f"vn{g}_{i}")
                      for i in range(OB)] for g in range(2)]
            vp16 = [pp.tile([128, VPW], bf16, name=f"vp16_{i}")
                    for i in range(SKB)]
            vp3 = [t.rearrange("p (h c) -> p h c", c=Dh + 1) for t in vp16]

            # ---- input DMA, priority order ----
            nc.sync.dma_start(out=wk16t, in_=wk16[:, :, :])
            for kb in range(DB):
                nc.sync.dma_start(out=y16t[:, kb, :], in_=y16[:, kb, :])
            nc.sync.dma_start(out=wq16t, in_=wq16[:, :, :])
            for kb in range(DB):
                nc.sync.dma_start(out=x16t[:, kb, :], in_=x16[:, kb, :])
            nc.sync.dma_start(out=wv16t, in_=wv16[:, :, :])
            nc.sync.dma_start(out=wo16t, in_=wo16[:, :, :])

            for h in range(HC):
                z0 = 64 if h % 2 == 0 else 0
                nc.gpsimd.memset(kTz[h][z0:z0 + 64, :], 0.0)
            for skb in range(SKB):
                nc.vector.memset(vp3[skb][:, :, Dh], 1.0)

            # ---- projection / output chunk emitters ----
            def emit_kt_chunk(ob, c4):
                ps = pj_ps.tile([128, 512], f32, name="pjps", tag="pjps")
                for kb in range(DB):
                    nc.tensor.matmul(
                        ps[:, :],
                        wk16t[:, kb, ob * 128:(ob + 1) * 128],
                        y16t[:, kb, c4 * 512:(c4 + 1) * 512],
                        start=(kb == 0), stop=(kb == DB - 1))
                cols = slice(c4 * 512, (c4 + 1) * 512)
                nc.vector.tensor_copy(kTz[2 * ob][0:64, cols], ps[0:64, :])
                nc.vector.tensor_copy(kTz[2 * ob + 1][64:128, cols],
                                      ps[64:128, :])

            def emit_qt_chunk(ob, c4):
                ps = pj_ps.tile([128, 512], f32, name="pjps", tag="pjps")
                for kb in range(DB):
                    nc.tensor.matmul(
                        ps[:, :],
                        wq16t[:, kb, ob * 128:(ob + 1) * 128],
                        x16t[:, kb, c4 * 512:(c4 + 1) * 512],
                        start=(kb == 0), stop=(kb == DB - 1))
                nc.vector.tensor_copy(
                    qT[ob][:, c4 * 512:(c4 + 1) * 512], ps[:, :])

            def emit_vp_chunk(skb):
                ps = pj_ps.tile([128, 512], f32, name="pjps", tag="pjps")
                for kb in range(DB):
                    nc.tensor.matmul(
                        ps[:, 0:DC],
                        y16t[:, kb, skb * 128:(skb + 1) * 128],
                        wv16t[:, kb, :],
                        start=(kb == 0), stop=(kb == DB - 1))
                src = ps[:, 0:DC].rearrange("p (h c) -> p h c", c=Dh)
                nc.vector.tensor_copy(vp3[skb][:, :, 0:Dh], src)

            o_stage = {}

            def emit_o_half(g, sqb, nc2):
                # half of a partial output-projection query block, using a
                # 1-bank pj slot (safe to interleave with live attention)
                n0, n1 = nc2 * 512, min(D, (nc2 + 1) * 512)
                ps = pj_ps.tile([128, 512], f32, name="pjps", tag="pjps")
                for kb in range(OB):
                    nc.tensor.matmul(
                        ps[:, 0:n1 - n0],
                        vnorm[g][kb][:, sqb * 128:(sqb + 1) * 128],
                        wo16t[:, kb, n0:n1],
                        start=(kb == 0), stop=(kb == OB - 1))
                if nc2 == 0:
                    o_stage[(g, sqb)] = o_pool.tile([128, D], f32,
                                                    name="osb")
                ot = o_stage[(g, sqb)]
                nc.vector.tensor_copy(ot[:, n0:n1], ps[:, 0:n1 - n0])
                if nc2 == 1:
                    o_stage.pop((g, sqb))
                    row0 = g * SQ + sqb * 128
                    nc.sync.dma_start(out=out[row0:row0 + 128, :],
                                      in_=ot[:, :])

            def emit_o_chunk(g, sqb):
                # tail-time version: big rotating slots (mm/valT free then)
                if sqb % 3 < 2:
                    op = mm_ps.tile([128, D], f32, name="mmps", tag="mmps",
                                    padded_shape=[128, SQ])
                else:
                    op = vt_ps.tile([128, D], f32, name="valT", tag="valT",
                                    padded_shape=[128, SQ])
                for kb in range(OB):
                    for nc2 in range(2):
                        n0, n1 = nc2 * 512, min(D, (nc2 + 1) * 512)
                        nc.tensor.matmul(
                            op[:, n0:n1],
                            vnorm[g][kb][:, sqb * 128:(sqb + 1) * 128],
                            wo16t[:, kb, n0:n1],
                            start=(kb == 0), stop=(kb == OB - 1))
                ot = o_pool.tile([128, D], f32, name="osb")
                nc.vector.tensor_copy(ot[:, :], op[:, :])
                row0 = g * SQ + sqb * 128
                nc.sync.dma_start(out=out[row0:row0 + 128, :], in_=ot[:, :])

            # task queue: (tag, mm_count, emit_fn), in need-by order
            tasks = []

            def queue_ob(ob):
                for c4 in range(4):
                    tasks.append((("kt", ob, c4), 6,
                                  lambda ob=ob, c4=c4: emit_kt_chunk(ob, c4)))

            for skb in range(SKB):
                tasks.append((("vp", skb), 6,
                              lambda skb=skb: emit_vp_chunk(skb)))
            queue_ob(1)
            for c4 in range(2, 4):  # qT[0] cols 1024-2047 (pass 1)
                tasks.append((("qt", 0, c4), 6,
                              lambda c4=c4: emit_qt_chunk(0, c4)))
            queue_ob(2)
            for ob in (1, 2):
                for c4 in range(4):
                    tasks.append((("qt", ob, c4), 6,
                                  lambda ob=ob, c4=c4: emit_qt_chunk(ob, c4)))

            state = {"budget": 0.0}

            def force(pred):
                rest = []
                for t in tasks:
                    if pred(t[0]):
                        t[2]()
                        state["budget"] -= t[1]
                    else:
                        rest.append(t)
                tasks[:] = rest

            def inject(budget_add):
                state["budget"] += budget_add
                while tasks and tasks[0][1] <= state["budget"]:
                    tag, mms, fn = tasks.pop(0)
                    fn()
                    state["budget"] -= mms

            # ---- prelude: kTz[0,1], then qT[0] cols 0-1023 ----
            for c4 in range(4):
                emit_kt_chunk(0, c4)
            for c4 in range(2):
                emit_qt_chunk(0, c4)

            # ---- attention pipeline over 12 (qhalf, head) pseudo-units ----
            pt_live = {}

            def emit_qk(g, h, s):
                hb = h // 2
                st = mm_ps.tile([128, SQ], f32, name="mmps", tag="mmps",
                                padded_shape=[128, SQ])
                for j in range(2):
                    q0 = g * SQ + j * 512
                    nc.tensor.matmul(
                        st[:, j * 512:(j + 1) * 512],
                        kTz[h][:, s * 128:(s + 1) * 128],
                        qT[hb][:, q0:q0 + 512],
                        start=True, stop=True)
                pt = pt_pool.tile([128, SQ], bf16, name="pt16")
                nc.scalar.activation(pt[:, :], st[:, :], EXP, scale=0.125)
                pt_live[(g, h, s)] = pt

            vt_live = {}

            def emit_pv(g, h, s):
                if s == 0:
                    vt_live[(g, h)] = vt_ps.tile([65, SQ], f32, name="valT",
                                                 tag="valT",
                                                 padded_shape=[128, SQ])
                vt = vt_live[(g, h)]
                pt = pt_live.pop((g, h, s))
                force(lambda t: t[0] == "vp" and t[1] == s)
                for j in range(2):
                    nc.tensor.matmul(
                        vt[:, j * 512:(j + 1) * 512],
                        vp16[s][:, h * 65:h * 65 + 65],
                        pt[:, j * 512:(j + 1) * 512],
                        start=(s == 0), stop=(s == SKB - 1))

            def emit_vnorm(g, h):
                hb, r0 = h // 2, (h % 2) * 64
                vt = vt_live.pop((g, h))
                vals = nrm_pool.tile([64, SQ], f32, name="vals")
                nc.vector.tensor_copy(vals[:, :], vt[0:64, :])
                den = nrm_pool.tile([1, SQ], f32, name="den")
                nc.vector.tensor_copy(den[:, :], vt[64:65, :])
                rec = nrm_pool.tile([1, SQ], f32, name="rec")
                # denominators are positive, well inside normal fp32 range;
                # approx-fast needs a partition-aligned source (den bounce)
                nc.vector.reciprocal_approx_fast(rec[:, :], den[:, :])
                rbc = nrm_pool.tile([64, SQ], f32, name="rbc")
                nc.gpsimd.partition_broadcast(rbc[:, :], rec[:, :])
                nc.vector.tensor_mul(
                    vnorm[g][hb][r0:r0 + 64, :], vals[:, :], rbc[:, :])

            units = [(g, h) for g in range(2) for h in range(HC)]
            NS = len(units) * SKB  # 192
            LAG = 3
            for u in range(NS + LAG):
                if u < NS:
                    uh, s2 = divmod(u, SKB)
                    g2, h2 = units[uh]
                    if s2 == 0 and h2 % 2 == 0:
                        # qT block hb2's columns for this query half
                        force(lambda t, hb2=h2 // 2, g=g2:
                              t[0] == "qt" and t[1] == hb2
                              and t[2] // 2 == g)
                    if s2 % 4 == 0:
                        force(lambda t, hb2=h2 // 2, c4=s2 // 4:
                              t[0] == "kt" and t[1] == hb2 and t[2] == c4)
                    emit_qk(g2, h2, s2)
                    inject(0.95)
                if u >= LAG:
                    uh, s1 = divmod(u - LAG, SKB)
                    g1, h1 = units[uh]
                    emit_pv(g1, h1, s1)
                    if s1 == SKB - 1:
                        emit_vnorm(g1, h1)
                        if h1 == HC - 1 and g1 == 0:
                            # pass 0 finished: queue its output projection
                            for sqb in range(8):
                                for nc2 in range(2):
                                    tasks.append((("o0", sqb), 3,
                                                  lambda sqb=sqb, nc2=nc2:
                                                  emit_o_half(0, sqb, nc2)))
                    inject(0.95)

            force(lambda t: True)

            # ---- pass 1 output projection ----
            for sqb in range(8):
                emit_o_chunk(1, sqb)

    nc.compile()
    return nc


def _get_nc():
    if "nc" not in _cache:
        _cache["nc"] = _build_nc()
    return _cache["nc"]


def _host_fallback(x, y, mask, Wq, bq, Wkv, bkv, Wo, bo):
    Bb, Ss, _ = x.shape
    q = x @ Wq.T + bq
    kv = y @ Wkv.T + bkv
    q = q.reshape(Bb, Ss, H, Dh).transpose(0, 2, 1, 3)
    kv = kv.reshape(Bb, Ss, H, 2 * Dh).transpose(0, 2, 1, 3)
    k, v = kv[..., :Dh], kv[..., Dh:]
    scaled = np.einsum("bhqd,bhkd->bhqk", q, k) / np.sqrt(np.float32(Dh))
    scaled = scaled + mask
    scaled -= scaled.max(axis=-1, keepdims=True)
    e = np.exp(scaled)
    attn = e / e.sum(axis=-1, keepdims=True)
    values = np.einsum("bhqk,bhkd->bhqd", attn, v)
    values = values.transpose(0, 2, 1, 3).reshape(Bb, Ss, H * Dh)
    return (values @ Wo.T + bo).astype(np.float32)


def _blk(mat_t, dtype):
    """[768, N] row-blocked to [128, 6, N] in the given ml dtype."""
    n = mat_t.shape[1]
    return np.ascontiguousarray(
        mat_t.reshape(-1, 128, n).transpose(1, 0, 2)).astype(dtype)


def _run(inputs, trace=False, trace_cores=None):
    """Returns (full_output, BassKernelResults)."""
    import ml_dtypes
    from concourse.bass_utils import run_bass_kernel_spmd

    bf16 = ml_dtypes.bfloat16

    x = np.ascontiguousarray(np.asarray(inputs["x"], dtype=np.float32))
    y = np.ascontiguousarray(np.asarray(inputs["y"], dtype=np.float32))
    Wq = np.asarray(inputs["Wq"], dtype=np.float32)
    Wkv = np.asarray(inputs["Wkv"], dtype=np.float32)
    Wo = np.asarray(inputs["Wo"], dtype=np.float32)

    # Reference reshapes kv to [B,S,H,2*Dh]: per head, rows h*128..h*128+63 of
    # Wkv are the k-projection, rows h*128+64..h*128+127 the v-projection.
    in_maps = []
    for c in range(N_CORES):
        b, g = c // 2, c % 2
        heads = range(g * HC, (g + 1) * HC)
        k_rows = np.concatenate([np.arange(h * 128, h * 128 + Dh)
                                 for h in heads])
        v_rows = np.concatenate([np.arange(h * 128 + Dh, (h + 1) * 128)
                                 for h in heads])
        q_rows = np.concatenate([np.arange(h * Dh, (h + 1) * Dh)
                                 for h in heads])
        in_maps.append({
            "x16": _blk(x[b].T, bf16),
            "y16": _blk(y[b].T, bf16),
            "wq16": _blk(Wq[q_rows].T, bf16),
            "wk16": _blk(Wkv[k_rows].T, bf16),
            "wv16": _blk(Wkv[v_rows].T, bf16),
            "wo16": _blk(Wo[:, q_rows].T, bf16),
        })

    nc = _get_nc()
    res = run_bass_kernel_spmd(nc, in_maps, core_ids=list(range(N_CORES)),
                               trace=trace, trace_cores=trace_cores)
    out = np.empty((B, S, D), dtype=np.float32)
    for b in range(B):
        # host-side "all-reduce" of the two head-halves' partial projections
        out[b] = res.results[2 * b]["out"]
        out[b] += res.results[2 * b + 1]["out"]
    return out, res


def kernel(**inputs) -> np.ndarray:
    mask = np.asarray(inputs["mask"], dtype=np.float32)
    bq = np.asarray(inputs["bq"], dtype=np.float32)
    bkv = np.asarray(inputs["bkv"], dtype=np.float32)
    bo = np.asarray(inputs["bo"], dtype=np.float32)
    if mask.any() or bq.any() or bkv.any() or bo.any():
        # Device kernel hardcodes zero mask/biases; stay correct regardless.
        return _host_fallback(
            np.asarray(inputs["x"], dtype=np.float32),
            np.asarray(inputs["y"], dtype=np.float32),
            mask, np.asarray(inputs["Wq"], dtype=np.float32), bq,
            np.asarray(inputs["Wkv"], dtype=np.float32), bkv,
            np.asarray(inputs["Wo"], dtype=np.float32), bo)
    out, _ = _run(inputs)
    return out
